# revision 11
# baseline (speedup 1.0000x reference)
"""DeepFRI GCN (3x GraphConv + mean-pool + MLP head) on 8 Trainium2 NeuronCores.

Key observation: the MLP head consumes only the mean-pooled graph
representation mean(concat([f1, f2, f3]), axis=0).  With
M = Din^{-1/2} A Dout^{-1/2} (the normalized aggregation operator) and
f_{k+1} = M f_k W_k + 1 b_k^T, the needed column sums collapse to

    1^T f1 = (u1^T x) W0 + n b0^T
    1^T f2 = (u2^T x) W0 W1 + s1 b0^T W1 + n b1^T
    1^T f3 = (u3^T x) W0 W1 W2 + s2 b0^T W1 W2 + s1 b1^T W2 + n b2^T

where u_k = (M^T)^k 1 are graph-only vectors (O(E) host index work, same
category as degree norms) and s_k = sum(u_k).  So the device only has to
 1. compute three u-weighted column sums of x  (nodes sharded 8-way),
 2. AllReduce the [3,1280] partials (15 KB),
 3. run three tiny matvec chains through W0/W1/W2 + the MLP head
    (replicated on every core).

Precision: colsum + chains in fp16 (fp32 PSUM accumulate), head in fp32r.
Measured sensitivity: fp16 head costs 1.5e-2 rel err, fp16 everywhere
else only ~4.5e-4 (tolerance 2e-2).

Layout note: compute engines require partition starts at 0 (BIR
verifier), so chain vectors are kept in the FREE dim (columns) wherever
single-row manipulation is needed: each layer's [3, D] psum rows are
copied full-tile to sbuf, PE-transposed in [3,128] blocks, and the
row permutations / row sums / bias adds happen as strided free-dim DVE
ops on the transposed columns, with bias terms precomputed host-side as
transposed column tensors.
"""

import numpy as np

P = 128
N_CORES = 8
D = 1280
NKI = D // P            # 10 k-chunks of 128
N_PAD = N_CORES * NKI * P   # 10240 padded nodes
RPC = N_PAD // N_CORES      # 1280 rows per core
OC_SIZES = (512, 512, 256)  # free-dim chunking of 1280 (psum bank = 2KB)
OC_OFFS = (0, 512, 1024)


# ---------------------------------------------------------------------------
# host-side graph preprocessing (pure index/degree work, like degree norms)
# ---------------------------------------------------------------------------

def preprocess(edge_index, n_nodes):
    src = np.asarray(edge_index[0], dtype=np.int64)
    dst = np.asarray(edge_index[1], dtype=np.int64)
    out_deg = np.bincount(src, minlength=n_nodes).astype(np.float64)
    in_deg = np.bincount(dst, minlength=n_nodes).astype(np.float64)
    oi = 1.0 / np.sqrt(np.clip(out_deg, 1.0, None))
    ii = 1.0 / np.sqrt(np.clip(in_deg, 1.0, None))

    ii_dst = ii[dst]

    def MT(v):  # (M^T v)[s] = oi[s] * sum_{e: src=s} ii[dst_e] * v[dst_e]
        return oi * np.bincount(src, weights=ii_dst * v[dst], minlength=n_nodes)

    u1 = MT(np.ones(n_nodes))
    u2 = MT(u1)
    u3 = MT(u2)
    return dict(
        u=np.stack([u1, u2, u3], axis=1),  # [n, 3] float64
        sig1=float(u1.sum()),
        sig2=float(u2.sum()),
    )


# ---------------------------------------------------------------------------
# numpy golden model of the exact collapsed algebra (for validation)
# ---------------------------------------------------------------------------

def golden(node_feat0, edge_index, n_nodes, W0, b0, W1, b1, W2, b2,
           Wh1, bh1, Wh2, bh2):
    n = int(n_nodes)
    pre = preprocess(edge_index, n)
    x = np.asarray(node_feat0, np.float64)
    W0, W1, W2 = (np.asarray(w, np.float64) for w in (W0, W1, W2))
    b0, b1, b2 = (np.asarray(b, np.float64) for b in (b0, b1, b2))
    S = pre["u"].T @ x  # [3, D]
    A1 = S @ W0
    c1 = A1[0] + n * b0
    Bp = np.stack([A1[1] + pre["sig1"] * b0, A1[2], pre["sig2"] * b0]) @ W1
    c2 = Bp[0] + n * b1
    c3 = (Bp[1] + Bp[2] + pre["sig1"] * b1) @ W2 + n * b2
    g = np.maximum(np.concatenate([c1, c2, c3]) / n, 0.0)
    h = np.maximum(g @ np.asarray(Wh1, np.float64) + np.asarray(bh1, np.float64), 0.0)
    return np.tanh(h @ np.asarray(Wh2, np.float64) + np.asarray(bh2, np.float64))


# ---------------------------------------------------------------------------
# Bass kernel (identical SPMD program on all 8 cores; data differs per core)
# ---------------------------------------------------------------------------

def build_nc():
    import concourse.bacc as bacc
    import concourse.mybir as mybir
    import concourse.tile as tile

    f32 = mybir.dt.float32
    f32r = mybir.dt.float32r
    f16 = mybir.dt.float16
    RELU = mybir.ActivationFunctionType.Relu
    TANH = mybir.ActivationFunctionType.Tanh

    nc = bacc.Bacc(
        "TRN2",
        target_bir_lowering=False,
        debug=False,
        num_devices=N_CORES,
    )

    # ---- kernel I/O (per-core contents, identical program)
    xs_d = nc.dram_tensor("xs", [P, NKI * D], f16, kind="ExternalInput")
    us_d = nc.dram_tensor("us", [P, NKI * 3], f16, kind="ExternalInput")
    w0_d = nc.dram_tensor("w0", [P, NKI * D], f16, kind="ExternalInput")
    w1_d = nc.dram_tensor("w1", [P, NKI * D], f16, kind="ExternalInput")
    w2_d = nc.dram_tensor("w2", [P, NKI * D], f16, kind="ExternalInput")
    wh1n_d = nc.dram_tensor("wh1n", [P, 3 * NKI * P], f32r, kind="ExternalInput")
    wh2_d = nc.dram_tensor("wh2", [P, 2], f32r, kind="ExternalInput")
    p16_d = nc.dram_tensor("p16", [P, NKI], f16, kind="ExternalInput")
    q16_d = nc.dram_tensor("q16", [P, NKI], f16, kind="ExternalInput")
    r16_d = nc.dram_tensor("r16", [P, NKI], f16, kind="ExternalInput")
    nbT_d = nc.dram_tensor("nbT", [P, NKI * 3], f32, kind="ExternalInput")
    bh1r_d = nc.dram_tensor("bh1r", [1, P], f32, kind="ExternalInput")
    bh2t_d = nc.dram_tensor("bh2t", [2, 1], f32, kind="ExternalInput")
    idT_d = nc.dram_tensor("idT", [3, 32], f32r, kind="ExternalInput")
    out_d = nc.dram_tensor("out", [2, 1], f32, kind="ExternalOutput")

    with tile.TileContext(nc) as tc:
        with (
            nc.allow_low_precision(reason="fp16 colsum/chains, fp32 head"),
            tc.tile_pool(name="dram", bufs=1, space="DRAM") as dram,
            tc.tile_pool(name="const", bufs=1) as const,
            tc.tile_pool(name="wpool", bufs=1) as wpool,
            tc.tile_pool(name="work", bufs=1) as work,
            tc.tile_pool(name="psA", bufs=1, space="PSUM") as psA,
            tc.tile_pool(name="psT", bufs=2, space="PSUM") as psT,
        ):
            sp_d = dram.tile([P, NKI * 3], f32, name="sp_d")
            st_d = dram.tile([P, NKI * 3], f32, name="st_d", addr_space="Shared")

            # ---- DMAs.  All big tensors are pre-swizzled on the host to
            # [128, N] sbuf layout so every DMA is a contiguous fat-descriptor
            # copy (12.8-25.6KB per partition run).  Ring assignment (rings
            # are FIFO, so latency-critical small transfers must not queue
            # behind the weight stream):
            #   SP ring:   x halves, sp_d write, AllReduce readback, out
            #   Act ring:  small constants, then w0, wh1n
            #   Pool ring (gpsimd swdge): w1, w2
            x_sb = const.tile([P, NKI, D], f16, name="x_sb")
            for h in range(2):
                nc.gpsimd.dma_start(
                    x_sb[:, h * 5 : (h + 1) * 5, :],
                    xs_d[:, h * 5 * D : (h + 1) * 5 * D],
                )
            u_sb = const.tile([P, NKI, 3], f16, name="u_sb")
            nc.scalar.dma_start(
                u_sb[:], us_d[:, :].rearrange("p (k r) -> p k r", r=3)
            )
            p16_sb = const.tile([P, NKI], f16, name="p16_sb")
            nc.scalar.dma_start(p16_sb[:], p16_d[:, :])
            q16_sb = const.tile([P, NKI], f16, name="q16_sb")
            nc.scalar.dma_start(q16_sb[:], q16_d[:, :])
            r16_sb = const.tile([P, NKI], f16, name="r16_sb")
            nc.scalar.dma_start(r16_sb[:], r16_d[:, :])
            nbT_sb = const.tile([P, NKI, 3], f32, name="nbT_sb")
            nc.scalar.dma_start(
                nbT_sb[:], nbT_d[:, :].rearrange("p (k r) -> p k r", r=3)
            )
            idT_sb = const.tile([3, 32], f32r, name="idT_sb")
            nc.scalar.dma_start(idT_sb[:], idT_d[:, :])
            bh1r_sb = const.tile([1, P], f32, name="bh1r_sb")
            nc.scalar.dma_start(bh1r_sb[:], bh1r_d[:, :])
            bh2t_sb = const.tile([2, 1], f32, name="bh2t_sb")
            nc.scalar.dma_start(bh2t_sb[:], bh2t_d[:, :])
            wh2_sb = const.tile([P, 2], f32r, name="wh2_sb")
            nc.scalar.dma_start(wh2_sb[:], wh2_d[:, :])

            def load_w(wd, name, dt, eng):
                wt = wpool.tile([P, NKI, D], dt, name=name)
                for h in range(2):
                    eng.dma_start(
                        wt[:, h * 5 : (h + 1) * 5, :],
                        wd[:, h * 5 * D : (h + 1) * 5 * D],
                    )
                return wt

            # Pool (SWDGE) ring saturates all 16 DMA engines (~345GB/s);
            # HWDGE rings cap around ~130GB/s.  Consumption-ordered stream on
            # Pool: x, w0, w1, w2h0; overflow (wh1n, w2h1) rides on Act
            # behind the small constants.
            w0_sb = load_w(w0_d, "w0_sb", f16, nc.gpsimd)
            w1_sb = load_w(w1_d, "w1_sb", f16, nc.gpsimd)
            w2_sb = wpool.tile([P, NKI, D], f16, name="w2_sb")
            nc.gpsimd.dma_start(w2_sb[:, 0:5, :], w2_d[:, 0 : 5 * D])
            nc.scalar.dma_start(w2_sb[:, 5:10, :], w2_d[:, 5 * D : 10 * D])
            wh1n_sb = wpool.tile([P, 3 * NKI, P], f32r, name="wh1n_sb")
            for h in range(2):
                nc.scalar.dma_start(
                    wh1n_sb[:, h * 15 : (h + 1) * 15, :],
                    wh1n_d[:, h * 15 * P : (h + 1) * 15 * P],
                )

            # ---- phase 1: S_part = U^T X over this core's node shard
            s_ps = [
                psA.tile([3, oc], f32, name=f"s_ps{i}", tag=f"a{i}")
                for i, oc in enumerate(OC_SIZES)
            ]
            for kc in range(NKI):
                for i, (oc, off) in enumerate(zip(OC_SIZES, OC_OFFS)):
                    nc.tensor.matmul(
                        s_ps[i][:, :],
                        u_sb[:, kc, :],
                        x_sb[:, kc, off : off + oc],
                        start=(kc == 0),
                        stop=(kc == NKI - 1),
                    )
            # transpose the [3, D] colsum rows to [128, NKI, 3] columns
            # on-chip so the collective payload and its readback are
            # contiguous DMAs (a strided row->col DMA costs ~8us in 4-byte
            # descriptors).
            sp_rows = work.tile([3, D], f32r, name="sp_rows")
            for i, (oc, off) in enumerate(zip(OC_SIZES, OC_OFFS)):
                nc.scalar.copy(sp_rows[:, off : off + oc], s_ps[i][:, :])
            scT_ps = psT.tile([P, NKI, 32], f32r, name="scT_ps", tag="tp")
            for ki in range(NKI):
                nc.tensor.transpose(
                    scT_ps[:, ki, :],
                    sp_rows[:, ki * P : (ki + 1) * P],
                    idT_sb[0:3, :],
                )
            spT_sb = work.tile([P, NKI, 3], f32, name="spT_sb")
            nc.scalar.copy(spT_sb[:], scT_ps[:, :, 0:3])
            nc.sync.dma_start(
                sp_d[:, :], spT_sb[:].rearrange("p k r -> p (k r)")
            )

            # ---- AllReduce the [3, D] partial colsums
            nc.gpsimd.collective_compute(
                "AllReduce",
                mybir.AluOpType.add,
                replica_groups=[list(range(N_CORES))],
                ins=[sp_d[:, :]],
                outs=[st_d[:, :]],
            )

            # readback: already in [p, k, r] column layout, contiguous
            a32T_sb = work.tile([P, NKI, 3], f32, name="a32T_sb")
            nc.sync.dma_start(
                a32T_sb[:], st_d[:, :].rearrange("p (k r) -> p k r", r=3)
            )
            a0T_sb = work.tile([P, NKI, 3], f16, name="a0T_sb")
            nc.scalar.copy(a0T_sb[:], a32T_sb[:])

            def chain(aT_ap_of_ki, w_sb, nv, lname):
                """nv-row matvec stack through one 1280x1280 weight."""
                f_ps = [
                    psA.tile([nv, oc], f32, name=f"f_{lname}{i}", tag=f"a{i}")
                    for i, oc in enumerate(OC_SIZES)
                ]
                for ki in range(NKI):
                    for i, (oc, off) in enumerate(zip(OC_SIZES, OC_OFFS)):
                        nc.tensor.matmul(
                            f_ps[i][:, :],
                            aT_ap_of_ki(ki),
                            w_sb[:, ki, off : off + oc],
                            start=(ki == 0),
                            stop=(ki == NKI - 1),
                        )
                # full-tile copy psum rows -> f32r sbuf rows (partition start 0)
                rows = work.tile([nv, D], f32r, name=f"rows_{lname}")
                for i, (oc, off) in enumerate(zip(OC_SIZES, OC_OFFS)):
                    nc.scalar.copy(rows[:, off : off + oc], f_ps[i][:, :])
                # PE-transpose [nv,128] blocks -> columns [128, nv] per chunk
                # PE ISA requires the moving operand free dim >= 32, so
                # the transpose "identity" is [nv, 32] (eye cols 0:nv, zeros
                # after) and each block lands in a 32-wide psum slot.
                cT = psT.tile([P, NKI, 32], f32r, name=f"cT_{lname}", tag="tp")
                for ki in range(NKI):
                    nc.tensor.transpose(
                        cT[:, ki, :],
                        rows[:, ki * P : (ki + 1) * P],
                        idT_sb[0:nv, :],
                    )
                # Return the psum view: each consumer DVE op below reads a
                # single strided psum column (one-psum-operand rule holds).
                return cT

            gT_sb = work.tile([P, NKI, 3], f32, name="gT_sb")
            bT_sb = work.tile([P, NKI, 3], f16, name="bT_sb")
            # constant column of B (sig2*b0) can be placed before the chains
            nc.vector.tensor_copy(bT_sb[:, :, 2], q16_sb[:, :])

            # ---- layer 0: A' = A @ W0 (A = [s1; s2; s3])
            t0 = chain(lambda ki: a0T_sb[:, ki, :], w0_sb, 3, "l0")
            # g col 0: c1 = A'[0] + n*b0
            nc.vector.tensor_add(gT_sb[:, :, 0], t0[:, :, 0], nbT_sb[:, :, 0])
            # B = [A'[1] + sig1*b0 ; A'[2] ; sig2*b0]  (fp16 columns)
            nc.vector.tensor_add(bT_sb[:, :, 0], t0[:, :, 1], p16_sb[:, :])
            nc.vector.tensor_copy(bT_sb[:, :, 1], t0[:, :, 2])

            # ---- layer 1: B' = B @ W1
            t1 = chain(lambda ki: bT_sb[:, ki, :], w1_sb, 3, "l1")
            # g col 1: c2 = B'[0] + n*b1
            nc.vector.tensor_add(gT_sb[:, :, 1], t1[:, :, 0], nbT_sb[:, :, 1])
            # C = B'[1] + B'[2] + sig1*b1  (fp16 columns)
            cT_sb = work.tile([P, NKI], f16, name="cT_sb")
            nc.vector.tensor_add(cT_sb[:, :], t1[:, :, 1], r16_sb[:, :])
            nc.vector.tensor_add(cT_sb[:, :], cT_sb[:, :], t1[:, :, 2])

            # ---- layer 2: c3 = C @ W2 + n*b2
            t2 = chain(lambda ki: cT_sb[:, ki : ki + 1], w2_sb, 1, "l2")
            nc.vector.tensor_add(gT_sb[:, :, 2], t2[:, :, 0], nbT_sb[:, :, 2])

            # ---- head: relu(g) @ (Wh1/n) + bh1, relu, @ Wh2 + bh2, tanh
            gr_sb = work.tile([P, NKI * 3], f32r, name="gr_sb")
            nc.scalar.activation(
                gr_sb[:], gT_sb[:].rearrange("p k r -> p (k r)"), RELU
            )
            h_ps = psT.tile([1, P], f32, name="h_ps", tag="tp")
            for l in range(3):
                for ki in range(NKI):
                    m = l * NKI + ki
                    nc.tensor.matmul(
                        h_ps[:, :],
                        gr_sb[:, ki * 3 + l : ki * 3 + l + 1],
                        wh1n_sb[:, m, :],
                        start=(m == 0),
                        stop=(m == 3 * NKI - 1),
                    )
            h2f_sb = work.tile([1, P], f32, name="h2f_sb")
            nc.vector.tensor_add(h2f_sb[:], h_ps[:, :], bh1r_sb[:])
            h2_sb = work.tile([1, P], f32r, name="h2_sb")
            nc.scalar.activation(h2_sb[:], h2f_sb[:], RELU)

            # h row -> column via padded transpose (cols 1:32 are zero)
            hT_ps = psT.tile([P, 32], f32r, name="hT_ps", tag="tp")
            nc.tensor.transpose(hT_ps[:, :], h2_sb[:, :], idT_sb[0:1, :])
            hT_sb = work.tile([P, 32], f32r, name="hT_sb")
            nc.scalar.copy(hT_sb[:], hT_ps[:])

            o_ps = psT.tile([2, 32], f32, name="o_ps", tag="tp")
            nc.tensor.matmul(
                o_ps[:, :], wh2_sb[:, :], hT_sb[:, :], start=True, stop=True
            )
            o_sb = work.tile([2, 1], f32, name="o_sb")
            nc.vector.tensor_add(o_sb[:], o_ps[:, 0:1], bh2t_sb[:])
            nc.scalar.activation(o_sb[:], o_sb[:], TANH)
            nc.sync.dma_start(out_d[:, :], o_sb[:])

    nc.compile()
    return nc


def _tcols(v):
    """[D] row vector -> [P, NKI] transposed column chunks."""
    return np.ascontiguousarray(np.asarray(v).reshape(NKI, P).T)


def _swz(w, nk):
    """[nk*128, N] row-major -> [128, nk*N] sbuf-layout pre-swizzle."""
    n = w.shape[1]
    return np.ascontiguousarray(
        w.reshape(nk, P, n).transpose(1, 0, 2).reshape(P, nk * n)
    )


def make_in_maps(inputs, pre):
    n = int(inputs["n_nodes"])
    x = np.asarray(inputs["node_feat0"], np.float32)
    W0 = np.asarray(inputs["W0"], np.float32)
    W1 = np.asarray(inputs["W1"], np.float32)
    W2 = np.asarray(inputs["W2"], np.float32)
    Wh1 = np.asarray(inputs["Wh1"], np.float32)
    Wh2 = np.asarray(inputs["Wh2"], np.float32)
    b0 = np.asarray(inputs["b0"], np.float32)
    b1 = np.asarray(inputs["b1"], np.float32)
    b2 = np.asarray(inputs["b2"], np.float32)
    bh1 = np.asarray(inputs["bh1"], np.float32)
    bh2 = np.asarray(inputs["bh2"], np.float32)
    sig1, sig2 = np.float32(pre["sig1"]), np.float32(pre["sig2"])

    # padded x / u shards
    x16 = np.zeros((N_PAD, D), np.float16)
    x16[:n] = x.astype(np.float16)
    u16 = np.zeros((N_PAD, 3), np.float16)
    u16[:n] = pre["u"].astype(np.float16)

    # nbT[p, k*3+l] = n * b_l[k*128+p]
    nbT = np.stack([_tcols(n * b0), _tcols(n * b1), _tcols(n * b2)], axis=2)

    common = dict(
        w0=_swz(W0.astype(np.float16), NKI),
        w1=_swz(W1.astype(np.float16), NKI),
        w2=_swz(W2.astype(np.float16), NKI),
        wh1n=_swz(Wh1 / np.float32(n), 3 * NKI),
        wh2=np.ascontiguousarray(Wh2),
        p16=_tcols(sig1 * b0).astype(np.float16),
        q16=_tcols(sig2 * b0).astype(np.float16),
        r16=_tcols(sig1 * b1).astype(np.float16),
        nbT=np.ascontiguousarray(nbT.reshape(P, NKI * 3)),
        bh1r=np.ascontiguousarray(bh1.reshape(1, P)),
        bh2t=np.ascontiguousarray(bh2.reshape(2, 1)),
        idT=np.ascontiguousarray(np.eye(3, 32, dtype=np.float32)),
    )
    in_maps = []
    for c in range(N_CORES):
        sl = slice(c * RPC, (c + 1) * RPC)
        m = dict(common)
        m["xs"] = _swz(x16[sl], NKI)
        # us[p, k*3+r] = u[c*RPC + k*128 + p, r]
        m["us"] = np.ascontiguousarray(
            u16[sl].reshape(NKI, P, 3).transpose(1, 0, 2).reshape(P, NKI * 3)
        )
        in_maps.append(m)
    return in_maps


last_results = None  # BassKernelResults of the most recent run (for test.py)


def kernel(**inputs):
    import os
    from concourse import bass_utils

    global last_results
    n = int(inputs["n_nodes"])
    pre = preprocess(inputs["edge_index"], n)
    nc = build_nc()
    in_maps = make_in_maps(inputs, pre)
    trace = os.environ.get("KERNEL_TRACE", "0") == "1"
    res = bass_utils.run_bass_kernel_spmd(
        nc, in_maps, core_ids=list(range(N_CORES)), trace=trace
    )
    last_results = res
    return np.asarray(res.results[0]["out"], np.float32).reshape(2)


if __name__ == "__main__":
    pass
